# revision 34
# baseline (speedup 1.0000x reference)
"""Trainium2 Bass kernel for nn_Circuit_71330816852913.

Physics: B=512 independent optical-cavity mode vectors A(t) in C^64 obeying
    dA/dt = A @ G + i * nl^2 * |A|^2 (.) A,   G = T2^T + i*diag(omega)
integrated over t in [0,1], sampled at 200 evenly spaced points (h = 1/199).

Method: windowed Strang splitting. The backbone advances the state in strides
of K=8 steps using two Strang segments of size 4h each (linear flow exact via
host-precomputed fp64 matrix exponentials applied as 128x128 real block
matmuls on the PE; the nonlinear flow is an exact elementwise phase rotation,
approximated by sin(x)~x, cos(x)~1-x^2/2 -- phase per substep is <=1e-2 rad).
The 7 intermediate outputs of each window are computed as INDEPENDENT single
Strang jumps of size j*h from the stride state: they hang off the latency
chain and only consume engine throughput. Splitting error O(tau^3) does not
accumulate for these branch jumps. Validated against the fp64 adaptive
reference on host: rel_l2 ~ 1.2e-5.

Sharding: pure data parallel over the batch dim, 64 rows per core on 8 cores.
On-chip layout per core: state stored transposed as [128 partitions, 64]
= [Re(A).T ; Im(A).T] (modes on partitions, batch on the free dim) so one
complex 64x64 matmul is a single PE matmul with a 128x128 stationary operand.
The re/im half-swap and the phi = |A|^2 half-sum are also constant 128x128
matmuls (PE is otherwise idle). All 200 output states accumulate in a SBUF
slab and ship to DRAM in 8 chunked DMAs overlapped with compute.
"""

import numpy as np

MODES = 64
INPUT_MODES = 48
LAMBD = 0.25
T_END = 1.0
EVAL_PTS = 200
N_CORES = 8
B_TOTAL = 512
BL = B_TOTAL // N_CORES  # 64 batch rows per core
NSTEP = EVAL_PTS - 1
H = T_END / NSTEP
KW = 8  # window stride


def _build_G(omega, kappa, params):
    """G (complex128 [64,64]) such that the linear RHS is A @ G for row-batch A."""
    n = MODES
    k = n * (n - 1) // 2
    p = params.astype(np.float64)
    diag_p = p[: n - 1]
    re = p[n - 1 : n - 1 + k]
    im = p[n - 1 + k :]
    Hm = np.zeros((n, n), np.complex128)
    iu, ju = np.triu_indices(n, 1)
    Hm[iu, ju] = re + 1j * im
    Hm = Hm + Hm.conj().T
    Hm = Hm + np.diag(np.concatenate([diag_p, [-diag_p.sum()]]))
    w, V = np.linalg.eigh(Hm)
    U = (V * np.exp(1j * w)[None, :]) @ V.conj().T
    I = np.eye(n)
    UtU = U.T @ U
    mix = UtU @ np.linalg.inv(I * (1.0 + LAMBD) - UtU)
    kap2 = (kappa.astype(np.float64).astype(np.complex128)) ** 2
    sk = np.sqrt(kap2)
    T2 = -(sk[:, None] * (0.5 * I + mix) * sk[None, :])
    return T2.T + 1j * np.diag(omega.astype(np.float64))


def _expm_series(X, terms=20):
    n = X.shape[0]
    E = np.eye(n, dtype=X.dtype)
    term = np.eye(n, dtype=X.dtype)
    for k in range(1, terms):
        term = term @ X / k
        E = E + term
    return E


_PROGRAM = None

# weight stack layout: idx i in 0..7 -> E(tau_i), idx 8+i -> SWAPS@E(tau_i)
# with tau_i = (i+1)*h/2 ; idx 16 -> SUM2 ; idx 16+j -> (nl^2*j*h)*SUM2 for
# j=1..7 (folds the per-branch phi scale into the half-sum matmul, since the
# Pool engine can only do plain tensor_tensor). Branch j uses idx j-1 / 8+j-1.
NW = 24
IDX_SUM2 = 16


def _get_program():
    global _PROGRAM
    if _PROGRAM is not None:
        return _PROGRAM

    import concourse.bacc as bacc
    import concourse.mybir as mybir
    import concourse.tile as tile
    from contextlib import ExitStack

    f32 = mybir.dt.float32
    Alu = mybir.AluOpType
    Act = mybir.ActivationFunctionType

    nc = bacc.Bacc(
        "TRN2", target_bir_lowering=False, debug=False, num_devices=N_CORES
    )
    y0_d = nc.declare_dram_parameter("y0", [128, 2 * BL], f32, isOutput=False)
    wts_d = nc.declare_dram_parameter("wts", [NW, 128, 128], f32, isOutput=False)
    sas_d = nc.declare_dram_parameter("sas", [128, 16], f32, isOutput=False)
    out_d = nc.declare_dram_parameter("out", [128, EVAL_PTS * BL], f32, isOutput=True)

    CHUNK = 25
    RSQRT2 = float(1.0 / np.sqrt(2.0))

    with ExitStack() as ctx:
        tc = ctx.enter_context(tile.TileContext(nc))
        const = ctx.enter_context(tc.tile_pool(name="const", bufs=1))
        work = ctx.enter_context(tc.tile_pool(name="work", bufs=6))
        slabp = ctx.enter_context(tc.tile_pool(name="slab", bufs=1))
        vwp = ctx.enter_context(tc.tile_pool(name="vwp", bufs=4, space="PSUM"))
        ppp = ctx.enter_context(tc.tile_pool(name="ppp", bufs=4, space="PSUM"))

        wsb = const.tile([128, NW * 128], f32, tag="wsb")
        for i in range(NW):
            nc.sync.dma_start(wsb[:, i * 128 : (i + 1) * 128], wts_d[i])
        sas_t = const.tile([128, 16], f32, tag="sas")
        nc.sync.dma_start(sas_t[:], sas_d[:])

        statep = ctx.enter_context(tc.tile_pool(name="statep", bufs=2))
        slab = slabp.tile([128, EVAL_PTS * BL], f32, tag="slab")
        state0 = statep.tile([128, 2 * BL], f32, tag="state", name="state0")
        nc.sync.dma_start(state0[:], y0_d[:])
        nc.sync.dma_start(slab[:, 0:BL], y0_d[:, 0:BL])

        def W(i):
            return wsb[:, i * 128 : (i + 1) * 128]

        # pre-observe every weight-slice DMA on the PE (fp32 self-loading
        # matmul tolerates only one sync wait)
        scratch = ppp.tile([1, 1], f32, tag="pp", name="scratch")
        for i in range(NW):
            nc.tensor.matmul(
                scratch[:],
                wsb[0:1, i * 128 : i * 128 + 1],
                wsb[0:1, i * 128 : i * 128 + 1],
                start=True,
                stop=True,
            )

        uid = [0]

        def stage_fh1(state_ap, eidx):
            """[V | psw] = E @ [Y | i*Y] in ONE matmul (E commutes with i)."""
            u = uid[0]
            uid[0] += 1
            vw = vwp.tile([128, 2 * BL], f32, tag="vw", name=f"vw{u}")
            nc.tensor.matmul(vw[:], W(eidx), state_ap, start=True, stop=True)
            psw_s = work.tile([128, BL], f32, tag="psw_s", name=f"psw_s{u}")
            nc.scalar.copy(psw_s[:], vw[:, BL : 2 * BL])
            return dict(u=u, vw=vw, psw_s=psw_s)

        def stage_fh1_two(src_ap, eidx):
            """V and psw via two matmuls from a plain [Y] source (backbone m2)."""
            u = uid[0]
            uid[0] += 1
            vw = vwp.tile([128, 2 * BL], f32, tag="vw", name=f"vw{u}")
            nc.tensor.matmul(vw[:, BL : 2 * BL], W(8 + eidx), src_ap, start=True, stop=True)
            psw_s = work.tile([128, BL], f32, tag="psw_s", name=f"psw_s{u}")
            nc.scalar.copy(psw_s[:], vw[:, BL : 2 * BL])
            nc.tensor.matmul(vw[:, 0:BL], W(eidx), src_ap, start=True, stop=True)
            return dict(u=u, vw=vw, psw_s=psw_s)

        def stage_fh2(st, sa_col, s_on_act):
            """S (squared, scaled) then phid = SUM2 @ S."""
            u = st["u"]
            S = work.tile([128, BL], f32, tag="S", name=f"S{u}")
            if s_on_act:
                nc.scalar.activation(
                    S[:], st["vw"][:, 0:BL], Act.Square, 0.0,
                    sas_t[:, 8 + sa_col : 9 + sa_col],
                )
            else:
                nc.gpsimd.tensor_mul(S[:], st["psw_s"][:], st["psw_s"][:])
            pp = ppp.tile([128, 2 * BL], f32, tag="pp", name=f"pp{u}")
            sum_idx = IDX_SUM2 if s_on_act else IDX_SUM2 + sa_col
            nc.tensor.matmul(pp[:, 0:BL], W(sum_idx), S[:], start=True, stop=True)
            st["pp"] = pp

        def stage_bh1(st):
            """q, t2, bb."""
            u = st["u"]
            q = work.tile([128, BL], f32, tag="q", name=f"q{u}")
            nc.scalar.activation(q[:], st["pp"][:, 0:BL], Act.Square, 0.0, RSQRT2)
            t2 = work.tile([128, BL], f32, tag="t2", name=f"t2{u}")
            nc.vector.tensor_mul(t2[:], st["pp"][:, 0:BL], st["psw_s"][:])
            bb = work.tile([128, BL], f32, tag="bb", name=f"bb{u}")
            nc.vector.scalar_tensor_tensor(
                bb[:], q[:], 1.0, st["vw"][:, 0:BL], Alu.subtract, Alu.mult
            )
            st["t2"], st["bb"] = t2, bb

        def stage_bh2(st, out_eidx, pos, cp_act):
            """nlo, output matmul, slab copy."""
            u = st["u"]
            nlo = work.tile([128, BL], f32, tag="nlo", name=f"nlo{u}")
            nc.gpsimd.tensor_sub(nlo[:], st["t2"][:], st["bb"][:])
            st["nlo"] = nlo
            if out_eidx is not None:
                pp = st["pp"]
                nc.tensor.matmul(
                    pp[:, BL : 2 * BL], W(out_eidx), nlo[:], start=True, stop=True
                )
                dst = slab[:, pos * BL : (pos + 1) * BL]
                if cp_act:
                    nc.scalar.copy(dst, pp[:, BL : 2 * BL])
                else:
                    nc.vector.tensor_copy(dst, pp[:, BL : 2 * BL])

        next_chunk = [0]

        def flush_chunks(done_through):
            while (next_chunk[0] + 1) * CHUNK - 1 <= done_through:
                c = next_chunk[0]
                lo, hi = c * CHUNK * BL, (c + 1) * CHUNK * BL
                nc.sync.dma_start(out_d[:, lo:hi], slab[:, lo:hi])
                next_chunk[0] += 1

        def emit_branches_pipelined(pstate, pkk, pt, bb_cl):
            """4-stage pipeline over branch units, with backbone closures
            (bb_cl) paced through the stream."""
            js = list(range(1, pkk))
            n = len(js)
            sts = {}
            total_ticks = n + 3
            nbb = len(bb_cl)
            marks = [int(i * total_ticks / max(nbb, 1)) for i in range(nbb)]
            bi = 0
            for tick in range(total_ticks):
                while bi < nbb and marks[bi] <= tick:
                    bb_cl[bi]()
                    bi += 1
                # deepest stage first
                i = tick - 3
                if 0 <= i < n:
                    j = js[i]
                    stage_bh2(sts.pop(j), j - 1, pt + j, False)
                i = tick - 2
                if 0 <= i < n:
                    stage_bh1(sts[js[i]])
                i = tick - 1
                if 0 <= i < n:
                    stage_fh2(sts[js[i]], js[i], False)
                if tick < n:
                    j = js[tick]
                    sts[j] = stage_fh1(pstate[:], j - 1)
            while bi < nbb:
                bb_cl[bi]()
                bi += 1

        # ---- window loop: backbone(w) paced through branches(w-1) ----
        prev = None
        state_t = state0
        t = 0
        while t < NSTEP:
            kk = min(KW, NSTEP - t)
            cur_state = state_t

            box = {}
            if kk == KW:

                def m1a(cur_state=cur_state):
                    box["s1"] = stage_fh1(cur_state[:], 3)
                    stage_fh2(box["s1"], 4, True)

                def m1b():
                    stage_bh1(box["s1"])
                    stage_bh2(box["s1"], None, None, True)

                def m2a():
                    box["s2"] = stage_fh1_two(box["s1"]["nlo"][:], 7)
                    stage_fh2(box["s2"], 4, True)

                def m2b(t=t, kk=kk):
                    # final: new [Y | i*Y] state, then slab copy of Y
                    st = box["s2"]
                    stage_bh1(st)
                    u = st["u"]
                    nlo = work.tile([128, BL], f32, tag="nlo", name=f"nlo{u}")
                    nc.gpsimd.tensor_sub(nlo[:], st["t2"][:], st["bb"][:])
                    pp = st["pp"]
                    nc.tensor.matmul(pp[:, 0:BL], W(3), nlo[:], start=True, stop=True)
                    nc.tensor.matmul(
                        pp[:, BL : 2 * BL], W(8 + 3), nlo[:], start=True, stop=True
                    )
                    ns = statep.tile([128, 2 * BL], f32, tag="state", name=f"state{u}")
                    nc.vector.tensor_copy(ns[:], pp[:])
                    pos = t + kk
                    nc.scalar.copy(slab[:, pos * BL : (pos + 1) * BL], pp[:, 0:BL])
                    box["ns"] = ns

                bb_cl = [m1a, m1b, m2a, m2b]
            else:

                def mfa(cur_state=cur_state, kk=kk):
                    box["s1"] = stage_fh1(cur_state[:], kk - 1)
                    stage_fh2(box["s1"], kk, True)

                def mfb(t=t, kk=kk):
                    stage_bh1(box["s1"])
                    stage_bh2(box["s1"], kk - 1, t + kk, True)

                bb_cl = [mfa, mfb]

            if prev is not None:
                pstate, pkk, pt = prev
                emit_branches_pipelined(pstate, pkk, pt, bb_cl)
                flush_chunks(pt + pkk - 1)
            else:
                for cl in bb_cl:
                    cl()
            prev = (cur_state, kk, t)
            if kk == KW:
                state_t = box["ns"]
            t += kk

        pstate, pkk, pt = prev
        emit_branches_pipelined(pstate, pkk, pt, [])
        flush_chunks(NSTEP)

    nc.finalize()
    _PROGRAM = nc
    return nc


def kernel(A0_real, A0_imag, omega, kappa, nonlinearity, params):
    from concourse.bass_utils import run_bass_kernel_spmd

    A0_real = np.asarray(A0_real, np.float32)
    A0_imag = np.asarray(A0_imag, np.float32)
    omega = np.asarray(omega, np.float32)
    kappa = np.asarray(kappa, np.float32)
    nonlinearity = np.asarray(nonlinearity, np.float32)
    params = np.asarray(params, np.float32)

    G = _build_G(omega, kappa, params)
    I64 = np.eye(64)
    Z64 = np.zeros((64, 64))
    SWAPS = np.block([[Z64, -I64], [I64, Z64]])
    SUM2 = np.block([[I64, I64], [I64, I64]])

    def real_block(C):
        return np.block([[C.real, -C.imag], [C.imag, C.real]])

    def lhsT(M):
        return np.ascontiguousarray(M.T).astype(np.float32)

    nl = float(nonlinearity.reshape(-1)[0])
    wts = np.zeros((NW, 128, 128), np.float32)
    for i in range(8):
        tau = (i + 1) * H / 2
        E = real_block(_expm_series(tau * G.T))
        wts[i] = lhsT(E)
        wts[8 + i] = lhsT(SWAPS @ E)
    wts[IDX_SUM2] = lhsT(SUM2)
    for j in range(1, 8):
        wts[IDX_SUM2 + j] = lhsT((nl * nl * j * H) * SUM2)

    # cols 0..7: unused ; cols 8..15: sa = |nl|*sqrt(j*h) (ACT path)
    sas = np.zeros((128, 16), np.float32)
    for j in range(1, 8):
        sas[:, j] = nl * nl * j * H
        sas[:, 8 + j] = abs(nl) * np.sqrt(j * H)

    Ar = np.concatenate(
        [A0_real, np.ones((B_TOTAL, MODES - INPUT_MODES), np.float32)], axis=1
    )
    Ai = np.concatenate(
        [A0_imag, np.zeros((B_TOTAL, MODES - INPUT_MODES), np.float32)], axis=1
    )
    Y0 = np.concatenate([Ar.T, Ai.T], axis=0).astype(np.float32)  # [128, 512]
    Y0sw = np.concatenate([-Y0[64:128], Y0[0:64]], axis=0).astype(np.float32)

    nc = _get_program()
    in_maps = []
    for c in range(N_CORES):
        in_maps.append(
            {
                "y0": np.ascontiguousarray(
                    np.concatenate(
                        [
                            Y0[:, c * BL : (c + 1) * BL],
                            Y0sw[:, c * BL : (c + 1) * BL],
                        ],
                        axis=1,
                    )
                ),
                "wts": wts,
                "sas": sas,
            }
        )
    res = run_bass_kernel_spmd(nc, in_maps, list(range(N_CORES)))

    parts = []
    for c in range(N_CORES):
        arr = np.asarray(res.results[c]["out"])  # [128, 200*64]
        parts.append(arr.reshape(2, 64, EVAL_PTS, BL).transpose(2, 0, 3, 1))
    out = np.concatenate(parts, axis=2)  # [200, 2, 512, 64]
    return np.ascontiguousarray(out.astype(np.float32))


# revision 38
# speedup vs baseline: 1.0102x; 1.0102x over previous
"""Trainium2 Bass kernel for nn_Circuit_71330816852913.

Physics: B=512 independent optical-cavity mode vectors A(t) in C^64 obeying
    dA/dt = A @ G + i * nl^2 * |A|^2 (.) A,   G = T2^T + i*diag(omega)
integrated over t in [0,1], sampled at 200 evenly spaced points (h = 1/199).

Method: windowed Strang splitting. The backbone advances the state in strides
of K=8 steps using two Strang segments of size 4h each (linear flow exact via
host-precomputed fp64 matrix exponentials applied as 128x128 real block
matmuls on the PE; the nonlinear flow is an exact elementwise phase rotation,
approximated by sin(x)~x, cos(x)~1-x^2/2 -- phase per substep is <=1e-2 rad).
The 7 intermediate outputs of each window are computed as INDEPENDENT single
Strang jumps of size j*h from the stride state: they hang off the latency
chain and only consume engine throughput. Splitting error O(tau^3) does not
accumulate for these branch jumps. Validated against the fp64 adaptive
reference on host: rel_l2 ~ 1.2e-5.

Sharding: pure data parallel over the batch dim, 64 rows per core on 8 cores.
On-chip layout per core: state stored transposed as [128 partitions, 64]
= [Re(A).T ; Im(A).T] (modes on partitions, batch on the free dim) so one
complex 64x64 matmul is a single PE matmul with a 128x128 stationary operand.
The re/im half-swap and the phi = |A|^2 half-sum are also constant 128x128
matmuls (PE is otherwise idle). All 200 output states accumulate in a SBUF
slab and ship to DRAM in 8 chunked DMAs overlapped with compute.
"""

import numpy as np

MODES = 64
INPUT_MODES = 48
LAMBD = 0.25
T_END = 1.0
EVAL_PTS = 200
N_CORES = 8
B_TOTAL = 512
BL = B_TOTAL // N_CORES  # 64 batch rows per core
NSTEP = EVAL_PTS - 1
H = T_END / NSTEP
KW = 8  # window stride


def _build_G(omega, kappa, params):
    """G (complex128 [64,64]) such that the linear RHS is A @ G for row-batch A."""
    n = MODES
    k = n * (n - 1) // 2
    p = params.astype(np.float64)
    diag_p = p[: n - 1]
    re = p[n - 1 : n - 1 + k]
    im = p[n - 1 + k :]
    Hm = np.zeros((n, n), np.complex128)
    iu, ju = np.triu_indices(n, 1)
    Hm[iu, ju] = re + 1j * im
    Hm = Hm + Hm.conj().T
    Hm = Hm + np.diag(np.concatenate([diag_p, [-diag_p.sum()]]))
    w, V = np.linalg.eigh(Hm)
    U = (V * np.exp(1j * w)[None, :]) @ V.conj().T
    I = np.eye(n)
    UtU = U.T @ U
    mix = UtU @ np.linalg.inv(I * (1.0 + LAMBD) - UtU)
    kap2 = (kappa.astype(np.float64).astype(np.complex128)) ** 2
    sk = np.sqrt(kap2)
    T2 = -(sk[:, None] * (0.5 * I + mix) * sk[None, :])
    return T2.T + 1j * np.diag(omega.astype(np.float64))


def _expm_series(X, terms=20):
    n = X.shape[0]
    E = np.eye(n, dtype=X.dtype)
    term = np.eye(n, dtype=X.dtype)
    for k in range(1, terms):
        term = term @ X / k
        E = E + term
    return E


_PROGRAM = None

# weight stack layout: idx i in 0..7 -> E(tau_i), idx 8+i -> SWAPS@E(tau_i)
# with tau_i = (i+1)*h/2 ; idx 16 -> SUM2 ; idx 16+j -> (nl^2*j*h)*SUM2 for
# j=1..7 (folds the per-branch phi scale into the half-sum matmul, since the
# Pool engine can only do plain tensor_tensor). Branch j uses idx j-1 / 8+j-1.
NW = 24
IDX_SUM2 = 16


def _get_program():
    global _PROGRAM
    if _PROGRAM is not None:
        return _PROGRAM

    import concourse.bacc as bacc
    import concourse.mybir as mybir
    import concourse.tile as tile
    from contextlib import ExitStack

    f32 = mybir.dt.float32
    Alu = mybir.AluOpType
    Act = mybir.ActivationFunctionType

    nc = bacc.Bacc(
        "TRN2", target_bir_lowering=False, debug=False, num_devices=N_CORES
    )
    y0_d = nc.declare_dram_parameter("y0", [128, 2 * BL], f32, isOutput=False)
    wts_d = nc.declare_dram_parameter("wts", [NW, 128, 128], f32, isOutput=False)
    sas_d = nc.declare_dram_parameter("sas", [128, 16], f32, isOutput=False)
    out_d = nc.declare_dram_parameter("out", [128, EVAL_PTS * BL], f32, isOutput=True)

    CHUNK = 25
    RSQRT2 = float(1.0 / np.sqrt(2.0))

    with ExitStack() as ctx:
        tc = ctx.enter_context(tile.TileContext(nc))
        const = ctx.enter_context(tc.tile_pool(name="const", bufs=1))
        work = ctx.enter_context(tc.tile_pool(name="work", bufs=6))
        slabp = ctx.enter_context(tc.tile_pool(name="slab", bufs=1))
        vwp = ctx.enter_context(tc.tile_pool(name="vwp", bufs=4, space="PSUM"))
        ppp = ctx.enter_context(tc.tile_pool(name="ppp", bufs=4, space="PSUM"))

        wsb = const.tile([128, NW * 128], f32, tag="wsb")
        for i in range(NW):
            nc.sync.dma_start(wsb[:, i * 128 : (i + 1) * 128], wts_d[i])
        sas_t = const.tile([128, 16], f32, tag="sas")
        nc.sync.dma_start(sas_t[:], sas_d[:])

        statep = ctx.enter_context(tc.tile_pool(name="statep", bufs=3))
        slab = slabp.tile([128, EVAL_PTS * BL], f32, tag="slab")
        state0 = statep.tile([128, 2 * BL], f32, tag="state", name="state0")
        nc.sync.dma_start(state0[:], y0_d[:])
        nc.sync.dma_start(slab[:, 0:BL], y0_d[:, 0:BL])

        def W(i):
            return wsb[:, i * 128 : (i + 1) * 128]

        # pre-observe every weight-slice DMA on the PE (fp32 self-loading
        # matmul tolerates only one sync wait)
        scratch = ppp.tile([1, 1], f32, tag="pp", name="scratch")
        for i in range(NW):
            nc.tensor.matmul(
                scratch[:],
                wsb[0:1, i * 128 : i * 128 + 1],
                wsb[0:1, i * 128 : i * 128 + 1],
                start=True,
                stop=True,
            )

        uid = [0]

        def stage_fh1(state_ap, eidx):
            """[V | psw] = E @ [Y | i*Y] in ONE matmul (E commutes with i)."""
            u = uid[0]
            uid[0] += 1
            vw = vwp.tile([128, 2 * BL], f32, tag="vw", name=f"vw{u}")
            nc.tensor.matmul(vw[:], W(eidx), state_ap, start=True, stop=True)
            psw_s = work.tile([128, BL], f32, tag="psw_s", name=f"psw_s{u}")
            nc.scalar.copy(psw_s[:], vw[:, BL : 2 * BL])
            return dict(u=u, vw=vw, psw_s=psw_s)

        def stage_fh1_two(src_ap, eidx):
            """V and psw via two matmuls from a plain [Y] source (backbone m2)."""
            u = uid[0]
            uid[0] += 1
            vw = vwp.tile([128, 2 * BL], f32, tag="vw", name=f"vw{u}")
            nc.tensor.matmul(vw[:, BL : 2 * BL], W(8 + eidx), src_ap, start=True, stop=True)
            psw_s = work.tile([128, BL], f32, tag="psw_s", name=f"psw_s{u}")
            nc.scalar.copy(psw_s[:], vw[:, BL : 2 * BL])
            nc.tensor.matmul(vw[:, 0:BL], W(eidx), src_ap, start=True, stop=True)
            return dict(u=u, vw=vw, psw_s=psw_s)

        def stage_fh2(st, sa_col, s_on_act):
            """S (squared, scaled) then phid = SUM2 @ S."""
            u = st["u"]
            S = work.tile([128, BL], f32, tag="S", name=f"S{u}")
            if s_on_act:
                nc.scalar.activation(
                    S[:], st["vw"][:, 0:BL], Act.Square, 0.0,
                    sas_t[:, 8 + sa_col : 9 + sa_col],
                )
            else:
                nc.gpsimd.tensor_mul(S[:], st["psw_s"][:], st["psw_s"][:])
            pp = ppp.tile([128, 2 * BL], f32, tag="pp", name=f"pp{u}")
            sum_idx = IDX_SUM2 if s_on_act else IDX_SUM2 + sa_col
            nc.tensor.matmul(pp[:, 0:BL], W(sum_idx), S[:], start=True, stop=True)
            st["pp"] = pp

        def stage_bh1(st):
            """q, t2, bb."""
            u = st["u"]
            q = work.tile([128, BL], f32, tag="q", name=f"q{u}")
            nc.scalar.activation(q[:], st["pp"][:, 0:BL], Act.Square, 0.0, RSQRT2)
            t2 = work.tile([128, BL], f32, tag="t2", name=f"t2{u}")
            nc.vector.tensor_mul(t2[:], st["pp"][:, 0:BL], st["psw_s"][:])
            bb = work.tile([128, BL], f32, tag="bb", name=f"bb{u}")
            nc.vector.scalar_tensor_tensor(
                bb[:], q[:], 1.0, st["vw"][:, 0:BL], Alu.subtract, Alu.mult
            )
            st["t2"], st["bb"] = t2, bb

        def stage_bh2(st, out_eidx, pos, cp_act):
            """nlo, output matmul, slab copy."""
            u = st["u"]
            nlo = work.tile([128, BL], f32, tag="nlo", name=f"nlo{u}")
            nc.gpsimd.tensor_sub(nlo[:], st["t2"][:], st["bb"][:])
            st["nlo"] = nlo
            if out_eidx is not None:
                pp = st["pp"]
                nc.tensor.matmul(
                    pp[:, BL : 2 * BL], W(out_eidx), nlo[:], start=True, stop=True
                )
                dst = slab[:, pos * BL : (pos + 1) * BL]
                if cp_act:
                    nc.scalar.copy(dst, pp[:, BL : 2 * BL])
                else:
                    nc.vector.tensor_copy(dst, pp[:, BL : 2 * BL])

        next_chunk = [0]

        def flush_chunks(done_through):
            while (next_chunk[0] + 1) * CHUNK - 1 <= done_through:
                c = next_chunk[0]
                lo, hi = c * CHUNK * BL, (c + 1) * CHUNK * BL
                nc.sync.dma_start(out_d[:, lo:hi], slab[:, lo:hi])
                next_chunk[0] += 1

        def emit_branches_pipelined(pstate, pkk, pt, bb_cl):
            """4-stage pipeline over branch units, with backbone closures
            (bb_cl) paced through the stream."""
            js = list(range(1, pkk))
            n = len(js)
            sts = {}
            total_ticks = n + 3
            nbb = len(bb_cl)
            marks = [int(i * total_ticks / max(nbb, 1)) for i in range(nbb)]
            bi = 0
            for tick in range(total_ticks):
                while bi < nbb and marks[bi] <= tick:
                    bb_cl[bi]()
                    bi += 1
                # deepest stage first
                i = tick - 3
                if 0 <= i < n:
                    j = js[i]
                    stage_bh2(sts.pop(j), j - 1, pt + j, False)
                i = tick - 2
                if 0 <= i < n:
                    stage_bh1(sts[js[i]])
                i = tick - 1
                if 0 <= i < n:
                    stage_fh2(sts[js[i]], js[i], False)
                if tick < n:
                    j = js[tick]
                    sts[j] = stage_fh1(pstate[:], j - 1)
            while bi < nbb:
                bb_cl[bi]()
                bi += 1

        # ---- window loop: backbone(w) paced through branches(w-1) ----
        prev = None
        state_t = state0
        t = 0
        while t < NSTEP:
            kk = min(KW, NSTEP - t)
            cur_state = state_t

            box = {}
            if kk == KW:

                def m1a(cur_state=cur_state):
                    box["s1"] = stage_fh1(cur_state[:], 3)
                    stage_fh2(box["s1"], 4, False)

                def m1b():
                    stage_bh1(box["s1"])
                    stage_bh2(box["s1"], None, None, True)

                def m2a():
                    box["s2"] = stage_fh1_two(box["s1"]["nlo"][:], 7)
                    stage_fh2(box["s2"], 4, False)

                def m2b(t=t, kk=kk):
                    # final: new [Y | i*Y] state, then slab copy of Y
                    st = box["s2"]
                    stage_bh1(st)
                    u = st["u"]
                    nlo = work.tile([128, BL], f32, tag="nlo", name=f"nlo{u}")
                    nc.gpsimd.tensor_sub(nlo[:], st["t2"][:], st["bb"][:])
                    pp = st["pp"]
                    nc.tensor.matmul(pp[:, 0:BL], W(3), nlo[:], start=True, stop=True)
                    nc.tensor.matmul(
                        pp[:, BL : 2 * BL], W(8 + 3), nlo[:], start=True, stop=True
                    )
                    ns = statep.tile([128, 2 * BL], f32, tag="state", name=f"state{u}")
                    nc.vector.tensor_copy(ns[:], pp[:])
                    pos = t + kk
                    nc.scalar.copy(slab[:, pos * BL : (pos + 1) * BL], pp[:, 0:BL])
                    box["ns"] = ns

                bb_cl = [m1a, m1b, m2a, m2b]
            else:

                def mfa(cur_state=cur_state, kk=kk):
                    box["s1"] = stage_fh1(cur_state[:], kk - 1)
                    stage_fh2(box["s1"], kk, False)

                def mfb(t=t, kk=kk):
                    stage_bh1(box["s1"])
                    stage_bh2(box["s1"], kk - 1, t + kk, True)

                bb_cl = [mfa, mfb]

            if prev is not None:
                pstate, pkk, pt = prev
                emit_branches_pipelined(pstate, pkk, pt, bb_cl)
                flush_chunks(pt + pkk - 1)
            else:
                for cl in bb_cl:
                    cl()
            prev = (cur_state, kk, t)
            if kk == KW:
                state_t = box["ns"]
            t += kk

        pstate, pkk, pt = prev
        emit_branches_pipelined(pstate, pkk, pt, [])
        flush_chunks(NSTEP)

    nc.finalize()
    _PROGRAM = nc
    return nc


def kernel(A0_real, A0_imag, omega, kappa, nonlinearity, params):
    from concourse.bass_utils import run_bass_kernel_spmd

    A0_real = np.asarray(A0_real, np.float32)
    A0_imag = np.asarray(A0_imag, np.float32)
    omega = np.asarray(omega, np.float32)
    kappa = np.asarray(kappa, np.float32)
    nonlinearity = np.asarray(nonlinearity, np.float32)
    params = np.asarray(params, np.float32)

    G = _build_G(omega, kappa, params)
    I64 = np.eye(64)
    Z64 = np.zeros((64, 64))
    SWAPS = np.block([[Z64, -I64], [I64, Z64]])
    SUM2 = np.block([[I64, I64], [I64, I64]])

    def real_block(C):
        return np.block([[C.real, -C.imag], [C.imag, C.real]])

    def lhsT(M):
        return np.ascontiguousarray(M.T).astype(np.float32)

    nl = float(nonlinearity.reshape(-1)[0])
    wts = np.zeros((NW, 128, 128), np.float32)
    for i in range(8):
        tau = (i + 1) * H / 2
        E = real_block(_expm_series(tau * G.T))
        wts[i] = lhsT(E)
        wts[8 + i] = lhsT(SWAPS @ E)
    wts[IDX_SUM2] = lhsT(SUM2)
    for j in range(1, 8):
        wts[IDX_SUM2 + j] = lhsT((nl * nl * j * H) * SUM2)

    # cols 0..7: unused ; cols 8..15: sa = |nl|*sqrt(j*h) (ACT path)
    sas = np.zeros((128, 16), np.float32)
    for j in range(1, 8):
        sas[:, j] = nl * nl * j * H
        sas[:, 8 + j] = abs(nl) * np.sqrt(j * H)

    Ar = np.concatenate(
        [A0_real, np.ones((B_TOTAL, MODES - INPUT_MODES), np.float32)], axis=1
    )
    Ai = np.concatenate(
        [A0_imag, np.zeros((B_TOTAL, MODES - INPUT_MODES), np.float32)], axis=1
    )
    Y0 = np.concatenate([Ar.T, Ai.T], axis=0).astype(np.float32)  # [128, 512]
    Y0sw = np.concatenate([-Y0[64:128], Y0[0:64]], axis=0).astype(np.float32)

    nc = _get_program()
    in_maps = []
    for c in range(N_CORES):
        in_maps.append(
            {
                "y0": np.ascontiguousarray(
                    np.concatenate(
                        [
                            Y0[:, c * BL : (c + 1) * BL],
                            Y0sw[:, c * BL : (c + 1) * BL],
                        ],
                        axis=1,
                    )
                ),
                "wts": wts,
                "sas": sas,
            }
        )
    res = run_bass_kernel_spmd(nc, in_maps, list(range(N_CORES)))

    parts = []
    for c in range(N_CORES):
        arr = np.asarray(res.results[c]["out"])  # [128, 200*64]
        parts.append(arr.reshape(2, 64, EVAL_PTS, BL).transpose(2, 0, 3, 1))
    out = np.concatenate(parts, axis=2)  # [200, 2, 512, 64]
    return np.ascontiguousarray(out.astype(np.float32))
